# revision 17
# baseline (speedup 1.0000x reference)
"""Trainium2 Bass kernel for nn_DilatedGraphConvolutionCell.

Structural facts (derived from the reference, hardcoded):
  - conv_layer output col t=10 is the only one read by Z0[:, :, -1]; for
    dilations 3 (layers 1..3) t=10 % 3 != 0 so Z1/Z2/Z3[:, :, -1] are exact
    zeros -> outputs 1..3 are host-side zeros.
  - out0 = relu(A(10,10) @ X10 @ (Wf0+Wb0) + b)
         + relu(A(9,10) @ X9 @ Wf1 + A(11,10) @ X9 @ Wb1 + b)
    where A(a,c) = softmax(threshold(U[:,:,a] @ B @ U[:,:,c].T)) row-wise
    (degree normalization is a no-op: softmax rows sum to 1).
  - X9/X10 = rows 9,10 of the FC stack (only 2 of 12 rows needed).
  - scores are in [-1.7, 0.2] so exp() needs no max-subtraction.

v3 design: bf16 weights/activations halve HBM traffic; two collectives
(AllGather of h2-partials + U column, AllGather of X). FC layers run in
stationary-weight form so activations come out pre-transposed (no PE
transpose chains) and biases are added as multi-partition tiles (DMA cost
on TRN2 scales with bytes-per-partition, so (1, N) bias rows are slow).
fc_w3 columns are host-permuted f-major so X lands node-major for the
(E @ X) @ W tail. Threshold/exp/rowsum work overlaps the second
AllGather.

Sharding over 8 cores:
  - fc_w1 cols /8 (h1 col-shard, local), fc_w2 rows /8 (partial h2),
    fc_w3 cols /8 (X node-shard 128/core).
  - AG1 = AllGather of [h2 partial^T (128,32) | U[:, :, 10]^T (12,128)].
  - AG2 = AllGather of X^T-packed (128,24) per core.
  - adjacency + output rows node-sharded (128 rows/core).
"""

import sys

sys.path.insert(0, "/opt/trn_rl_repo")

import numpy as np
import ml_dtypes

import concourse.bass as bass
import concourse.bacc as bacc
import concourse.tile as tile
from concourse import mybir
from concourse.bass_utils import run_bass_kernel_spmd
from concourse.masks import make_identity

F32 = mybir.dt.float32
BF16 = mybir.dt.bfloat16
NPBF = ml_dtypes.bfloat16
NCORES = 8
N = 1024  # nodes
F = 12    # features (== lookback)
H = 2048  # fc hidden
HS = H // NCORES        # 256  per-core shard of fc hidden
NPC = N // NCORES       # 128  nodes per core
W3S = NPC * F           # 1536 per-core cols of fc_w3
G1 = 256                # graph-stack hidden
GCOLS = NPC * 3         # 384  per-core needed cols of gs_w2/gt_w2
OBS_O = 0
LI_O = 128
GS2_O = 224
GT2_O = 992
B1_O = 1760
B2_O = 1764
B3_O = 1796
GS1_O = 1820
PK128W = 3868
TF_O = 256
GSB1_O = 257
GSB2_O = 513
GTB1_O = 897
GTB2_O = 1153
WF1_O = 1537
WB1_O = 1549
W01_O = 1561
BM_O = 1573
PK36W = 1585
AF = mybir.ActivationFunctionType
ALU = mybir.AluOpType
AX = mybir.AxisListType

_CACHE = {}


def _build():
    nc = bacc.Bacc("TRN2", target_bir_lowering=False, debug=False,
                   num_devices=NCORES)
    groups = [list(range(NCORES))]

    # ---- per-core external inputs (host supplies per-core slices) ----
    d_pk128 = nc.dram_tensor("pk128", [128, PK128W], BF16,
                             kind="ExternalInput")
    d_pk36 = nc.dram_tensor("pk36", [36, PK36W], BF16, kind="ExternalInput")
    d_w1 = nc.dram_tensor("w1_pack", [8, 128, 2048], BF16,
                          kind="ExternalInput")
    d_w2 = nc.dram_tensor("w2_pack", [2, 128, 2048], BF16,
                          kind="ExternalInput")
    d_w3 = nc.dram_tensor("w3c", [16, 128, W3S], BF16, kind="ExternalInput")

    d_out = nc.dram_tensor("out0", [NPC, F], F32, kind="ExternalOutput")

    with tile.TileContext(nc) as tc:
        with (
            tc.tile_pool(name="consts", bufs=1) as consts,
            tc.tile_pool(name="wbig", bufs=1) as wbig,
            tc.tile_pool(name="w3pool", bufs=1) as w3pool,
            tc.tile_pool(name="work", bufs=2) as work,
            tc.tile_pool(name="emat", bufs=1) as emat,
            tc.tile_pool(name="ps_acc", bufs=1, space="PSUM") as ps_acc,
            tc.tile_pool(name="ps_sm", bufs=2, space="PSUM") as ps_sm,
            tc.tile_pool(name="ps_big", bufs=2, space="PSUM") as ps_big,
            tc.tile_pool(name="ps_q", bufs=3, space="PSUM") as ps_q,
            tc.tile_pool(name="dram", bufs=1, space="DRAM") as dram,
        ):
            ident_bf = consts.tile([128, 128], BF16)
            make_identity(nc, ident_bf[:])
            ones_bf = consts.tile([1, 128], BF16)
            nc.vector.memset(ones_bf[:], 1.0)
            onescol_bf = consts.tile([128, 1], BF16)
            nc.vector.memset(onescol_bf[:], 1.0)

            # ---- packed consts: two DMAs instead of ~24 -----------------
            pk128_t = consts.tile([128, PK128W], BF16)
            nc.sync.dma_start(pk128_t[:], d_pk128[:])
            pk36_t = consts.tile([36, PK36W], BF16)
            nc.sync.dma_start(pk36_t[:], d_pk36[:])
            pv = pk128_t[:]
            obs_t = pv[:, OBS_O:OBS_O + 128]
            li_t = pv[:, LI_O:LI_O + 96]
            gs2_t = [pv[:, GS2_O + GCOLS * k:GS2_O + GCOLS * (k + 1)]
                     for k in range(2)]
            gt2_t = [pv[:, GT2_O + GCOLS * k:GT2_O + GCOLS * (k + 1)]
                     for k in range(2)]
            b1T_t = pv[:, B1_O:B1_O + 4]
            b2T_t = pv[:, B2_O:B2_O + 32]
            b3T_t = pv[:, B3_O:B3_O + 2 * F]
            gs1_t = pv[:, GS1_O:GS1_O + 2048]
            qv = pk36_t[:]
            gt1_t = qv[:, 0:G1]
            tf_t = qv[:, TF_O:TF_O + 1]
            gsb1_t = qv[0:1, GSB1_O:GSB1_O + G1]
            gsb2_t = qv[0:1, GSB2_O:GSB2_O + GCOLS]
            gtb1_t = qv[0:1, GTB1_O:GTB1_O + G1]
            gtb2_t = qv[0:1, GTB2_O:GTB2_O + GCOLS]
            Wf1b_t = qv[0:13, WF1_O:WF1_O + F]
            Wb1b_t = qv[0:13, WB1_O:WB1_O + F]
            W01b_t = qv[0:13, W01_O:W01_O + F]
            B_t = qv[0:12, BM_O:BM_O + F]
            # ---------------- graph embedding U --------------------------
            g1_ps = ps_acc.tile([F, G1], F32, tag="acc")
            for k in range(8):
                nc.tensor.matmul(g1_ps[:], li_t[:, 12 * k:12 * k + 12],
                                 gs1_t[:, 256 * k:256 * (k + 1)],
                                 start=(k == 0), stop=False)
            nc.tensor.matmul(g1_ps[:], ones_bf[:, :F], gsb1_t,
                             start=False, stop=True)
            g1_sb = work.tile([F, G1], BF16)
            nc.scalar.activation(g1_sb[:], g1_ps[:], AF.Relu)
            g1T_sb = work.tile([128, 24], BF16)
            for m in range(2):
                tp_ps = ps_sm.tile([128, F], BF16, tag="sm")
                nc.tensor.transpose(tp_ps[:], g1_sb[:, 128 * m:128 * (m + 1)],
                                    ident_bf[:F, :F])
                nc.vector.tensor_copy(g1T_sb[:, 12 * m:12 * (m + 1)], tp_ps[:])

            sp_ps = ps_acc.tile([F, GCOLS], F32, tag="acc")
            for k in range(2):
                nc.tensor.matmul(sp_ps[:], g1T_sb[:, 12 * k:12 * (k + 1)],
                                 gs2_t[k], start=(k == 0), stop=False)
            nc.tensor.matmul(sp_ps[:], ones_bf[:, :F], gsb2_t,
                             start=False, stop=True)
            sp_sb = work.tile([F, GCOLS], BF16)
            nc.scalar.activation(sp_sb[:], sp_ps[:], AF.Relu)

            t1_ps = ps_sm.tile([1, G1], F32, tag="sm")
            nc.tensor.matmul(t1_ps[:], tf_t, gt1_t, start=True,
                             stop=False)
            nc.tensor.matmul(t1_ps[:], ones_bf[:, :1], gtb1_t,
                             start=False, stop=True)
            t1_sb = work.tile([1, G1], BF16)
            nc.scalar.activation(t1_sb[:], t1_ps[:], AF.Relu)
            t1T_sb = work.tile([128, 2], BF16)
            for m in range(2):
                tt_ps = ps_sm.tile([128, 1], BF16, tag="sm")
                nc.tensor.transpose(tt_ps[:], t1_sb[:, 128 * m:128 * (m + 1)],
                                    ident_bf[:1, :1])
                nc.vector.tensor_copy(t1T_sb[:, m:m + 1], tt_ps[:])
            tp_ps2 = ps_acc.tile([1, GCOLS], F32, tag="acc")
            for k in range(2):
                nc.tensor.matmul(tp_ps2[:], t1T_sb[:, k:k + 1], gt2_t[k],
                                 start=(k == 0), stop=False)
            nc.tensor.matmul(tp_ps2[:], ones_bf[:, :1], gtb2_t,
                             start=False, stop=True)
            tp_sb = work.tile([1, GCOLS], BF16)
            nc.scalar.activation(tp_sb[:], tp_ps2[:], AF.Relu)
            tpb_ps = ps_sm.tile([F, GCOLS], F32, tag="sm")
            nc.tensor.matmul(tpb_ps[:], ones_bf[:, :F], tp_sb[:],
                             start=True, stop=True)
            U_sb = emat.tile([F, GCOLS], BF16)
            nc.vector.tensor_add(U_sb[:], sp_sb[:], tpb_ps[:])
            U_view = U_sb[:].rearrange("l (i s) -> l s i", s=3)

            # bu_s = B^T @ U1my^T for a = 9, 10, 11  (12, 128) each
            bu_sb = emat.tile([F, 3 * 128], BF16)
            for s in range(3):
                bps = ps_sm.tile([F, 128], F32, tag="sm")
                nc.tensor.matmul(bps[:], B_t, U_view[:, s, :],
                                 start=True, stop=True)
                nc.vector.tensor_copy(bu_sb[:, 128 * s:128 * (s + 1)], bps[:])

            # ---------------- fc1 (stationary w1, h1 lands transposed) ---
            h1_ps = ps_acc.tile([128, 4], F32, tag="acc")
            w1_tl = []
            for g in range(8):
                w1_t = wbig.tile([128, 2048], BF16, name=f"w1t{g}")
                nc.sync.dma_start(w1_t[:], d_w1[g])
                w1_tl.append(w1_t)
            w2_t = []
            for k in range(2):
                t = consts.tile([128, 2048], BF16, name=f"w2t{k}")
                nc.sync.dma_start(t[:, :1024], d_w2[k][:, :1024])
                nc.sync.dma_start(t[:, 1024:], d_w2[k][:, 1024:])
                w2_t.append(t)
            for m in range(2):
                for k in range(64):
                    g, kk = k // 8, k % 8
                    nc.tensor.matmul(
                        h1_ps[:, 2 * m:2 * m + 2],
                        w1_tl[g][:, 256 * kk + 128 * m:
                                 256 * kk + 128 * (m + 1)],
                        obs_t[:, 2 * k:2 * k + 2],
                        start=(k == 0), stop=(k == 63))
            h1f_sb = work.tile([128, 4], F32)
            nc.vector.tensor_add(h1f_sb[:], h1_ps[:], b1T_t)
            h1T_sb = work.tile([128, 4], BF16)
            nc.scalar.activation(h1T_sb[:], h1f_sb[:], AF.Relu)

            # ---------------- fc2 partial (transposed) -------------------
            h2p_ps = ps_acc.tile([128, 32], F32, tag="acc")
            for jk in range(16):
                for k in range(2):
                    nc.tensor.matmul(
                        h2p_ps[:, 2 * jk:2 * jk + 2],
                        w2_t[k][:, 128 * jk:128 * (jk + 1)],
                        h1T_sb[:, 2 * k:2 * k + 2],
                        start=(k == 0), stop=(k == 1))
            h2pT_sb = work.tile([128, 32], BF16)
            nc.scalar.activation(h2pT_sb[:], h2p_ps[:], AF.Copy)

            # ---------------- AG1: h2 partial^T + U col 10 ---------------
            AG1W = 128 * 32 + F * 128  # 5632 bf16 elements per core
            ag1_in = dram.tile([1, AG1W], BF16)
            nc.sync.dma_start(
                ag1_in[:, :4096].rearrange("a (p j) -> p (a j)", p=128),
                h2pT_sb[:])
            nc.sync.dma_start(
                ag1_in[:, 4096:].rearrange("a (l i) -> l (a i)", l=F),
                U_view[:, 1, :])
            ag1_out = dram.tile([NCORES, AG1W], BF16, addr_space="Shared")
            nc.gpsimd.collective_compute(
                "AllGather", ALU.bypass, replica_groups=groups,
                ins=[ag1_in[:].opt()], outs=[ag1_out[:].opt()])

            # ---------------- post-AG1: h2 reduce + U2T ------------------
            h2gT_sb = work.tile([128, 8 * 32], BF16)
            nc.sync.dma_start(
                h2gT_sb[:].rearrange("p (c j) -> p c j", c=8),
                ag1_out[:, :4096].rearrange("c (p j) -> p c j", p=128))
            U2T_sb = emat.tile([F, N], BF16)
            nc.sync.dma_start(
                U2T_sb[:].rearrange("l (c i) -> l c i", c=NCORES),
                ag1_out[:, 4096:].rearrange("c (l i) -> l c i", l=F))

            h2r_sb = work.tile([128, 32], F32)
            nc.vector.reduce_sum(
                h2r_sb[:],
                h2gT_sb[:].rearrange("p (c j) -> p j c", c=8),
                axis=AX.X)
            nc.vector.tensor_add(h2r_sb[:], h2r_sb[:], b2T_t)
            h2T_sb = work.tile([128, 32], BF16)
            nc.scalar.activation(h2T_sb[:], h2r_sb[:], AF.Relu)

            # ---------------- fc3 (stationary w3, X lands node-major) ----
            # w3 cols host-permuted to (f, node): h3T psum col = 2f + t
            h3_ps = ps_acc.tile([128, 2 * F], F32, tag="acc")
            w3_tl = []
            for k in range(16):
                w3_t = w3pool.tile([128, W3S], BF16, name=f"w3t{k}")
                nc.sync.dma_start(w3_t[:, :W3S // 2], d_w3[k][:, :W3S // 2])
                nc.sync.dma_start(w3_t[:, W3S // 2:], d_w3[k][:, W3S // 2:])
                w3_tl.append(w3_t)
            for f in range(F):
                for k in range(16):
                    nc.tensor.matmul(
                        h3_ps[:, 2 * f:2 * f + 2],
                        w3_tl[k][:, 128 * f:128 * (f + 1)],
                        h2T_sb[:, 2 * k:2 * k + 2],
                        start=(k == 0), stop=(k == 15))
            h3f_sb = work.tile([128, 2 * F], F32)
            nc.vector.tensor_add(h3f_sb[:], h3_ps[:], b3T_t)
            h3T_sb = work.tile([128, 2 * F], BF16)
            nc.scalar.activation(h3T_sb[:], h3f_sb[:], AF.Relu)

            # ---------------- AG2: X chunks (node-major) -----------------
            ag2_in = dram.tile([1, 128 * 2 * F], BF16)
            nc.sync.dma_start(
                ag2_in[:].rearrange("a (p j) -> p (a j)", p=128),
                h3T_sb[:])
            ag2_out = dram.tile([NCORES, 128 * 2 * F], BF16,
                                addr_space="Shared")
            nc.gpsimd.collective_compute(
                "AllGather", ALU.bypass, replica_groups=groups,
                ins=[ag2_in[:].opt()], outs=[ag2_out[:].opt()])

            # ---------------- E^T + rowsums (overlap AG2) ----------------
            E_sb = []
            for s in range(3):
                e_t = emat.tile([128, N], BF16, name=f"E{s}")
                E_sb.append(e_t)
            for s in range(3):
                for hh in range(2):
                    st_ps = ps_big.tile([128, 512], F32, tag="big")
                    for j in range(4):
                        kb = 4 * hh + j
                        nc.tensor.matmul(
                            st_ps[:, 128 * j:128 * (j + 1)],
                            U2T_sb[:, 128 * kb:128 * (kb + 1)],
                            bu_sb[:, 128 * s:128 * (s + 1)],
                            start=True, stop=True)
                    msk_sb = work.tile([128, 512], F32, tag="msk")
                    nc.vector.tensor_scalar(msk_sb[:], st_ps[:], 0.05, None,
                                            op0=ALU.is_ge)
                    xt2_sb = work.tile([128, 512], F32, tag="xt2")
                    nc.vector.tensor_mul(xt2_sb[:], msk_sb[:], st_ps[:])
                    nc.scalar.activation(
                        E_sb[s][:, 512 * hh:512 * (hh + 1)], xt2_sb[:],
                        AF.Exp)
            # rowsums rs_j = sum_i E[i, j] and reciprocals
            rinv_sb = []
            for s in range(3):
                rs_ps = ps_q.tile([128, 1], F32, name=f"rs{s}", tag="q")
                for k in range(8):
                    nc.tensor.matmul(rs_ps[:],
                                     E_sb[s][:, 128 * k:128 * (k + 1)],
                                     onescol_bf[:],
                                     start=(k == 0), stop=(k == 7))
                rv = work.tile([128, 1], F32, name=f"rinv{s}")
                nc.vector.reciprocal(rv[:], rs_ps[:])
                rinv_sb.append(rv)

            # ---------------- tail: R = E @ X, out = (R/rs) @ W ----------
            X_sb = emat.tile([128, NCORES * 2 * F], BF16)
            nc.sync.dma_start(
                X_sb[:].rearrange("p (c j) -> p c j", c=NCORES),
                ag2_out[:].rearrange("c (p j) -> p c j", p=128))
            X_v = X_sb[:].rearrange("p (c f t) -> p c t f", c=NCORES, f=F)

            qnT_sb = emat.tile([13, 3 * 128], BF16)
            qnT_v = qnT_sb[:]
            for s, t in ((0, 0), (1, 1), (2, 0)):
                r_ps = ps_q.tile([128, F], F32, name=f"r{s}", tag="q")
                for k in range(8):
                    nc.tensor.matmul(r_ps[:],
                                     E_sb[s][:, 128 * k:128 * (k + 1)],
                                     X_v[:, k, t, :],
                                     start=(k == 0), stop=(k == 7))
                qn_sb = work.tile([128, F + 1], BF16, name=f"qn{s}")
                nc.vector.memset(qn_sb[:, F:F + 1], 1.0)
                nc.vector.tensor_scalar_mul(qn_sb[:, :F], r_ps[:],
                                            rinv_sb[s][:])
                qt_ps = ps_sm.tile([F + 1, 128], BF16, tag="sm")
                nc.tensor.transpose(qt_ps[:], qn_sb[:], ident_bf[:])
                nc.vector.tensor_copy(qnT_v[:, 128 * s:128 * (s + 1)],
                                      qt_ps[:])

            # bias rides as row 12 of the augmented weights
            fin_ps = ps_q.tile([128, 2 * F], F32, name="fin", tag="q")
            nc.tensor.matmul(fin_ps[:, :F], qnT_v[:, 0:128], Wf1b_t,
                             start=True, stop=False)
            nc.tensor.matmul(fin_ps[:, :F], qnT_v[:, 256:384], Wb1b_t,
                             start=False, stop=True)
            nc.tensor.matmul(fin_ps[:, F:], qnT_v[:, 128:256], W01b_t,
                             start=True, stop=True)
            fin_sb = work.tile([128, 2 * F], F32)
            nc.scalar.activation(fin_sb[:], fin_ps[:], AF.Relu)
            out_sb = work.tile([128, F], F32)
            nc.vector.tensor_add(out_sb[:], fin_sb[:, :F], fin_sb[:, F:])
            nc.sync.dma_start(d_out[:], out_sb[:])

    nc.compile()
    return nc


def _prep_inputs(inputs):
    """Host-side slicing/packing of the full inputs into per-core maps."""
    f32 = np.float32
    bf = NPBF
    obs = np.asarray(inputs["observation"], f32)
    obs2T = np.stack([obs[:, :, 9].reshape(-1), obs[:, :, 10].reshape(-1)],
                     axis=1)                       # (8192, 2)
    obs_pack = np.ascontiguousarray(
        obs2T.reshape(64, 128, 2).transpose(1, 0, 2).reshape(128, 128))
    li = np.asarray(inputs["layer_initial"], f32)   # (12, 1024)
    li_pack = np.ascontiguousarray(
        li.T.reshape(8, 128, 12).transpose(1, 0, 2).reshape(128, 96))
    gs1_pack = np.ascontiguousarray(
        np.asarray(inputs["gs_w1"], f32).reshape(8, 128, G1)
        .transpose(1, 0, 2).reshape(128, 2048))
    w1 = np.asarray(inputs["fc_w1"], f32)
    w2 = np.asarray(inputs["fc_w2"], f32)
    w3 = np.asarray(inputs["fc_w3"], f32)
    b1 = np.asarray(inputs["fc_b1"], f32)
    b2 = np.asarray(inputs["fc_b2"], f32)
    b3 = np.asarray(inputs["fc_b3"], f32)
    bv = np.asarray(inputs["b"], f32)
    gs2 = np.asarray(inputs["gs_w2"], f32)
    gt2 = np.asarray(inputs["gt_w2"], f32)
    Wf = np.asarray(inputs["W_forward"], f32)
    Wb = np.asarray(inputs["W_backward"], f32)
    b2T = np.repeat(b2.reshape(16, 128).T, 2, axis=1)  # (128, 32)

    # pk36: rows 0-35; gt1 | tf | row-0 biases | 13-row aug weights | B
    pk36 = np.zeros((36, PK36W), f32)
    pk36[:, 0:G1] = np.asarray(inputs["gt_w1"], f32)
    pk36[:, TF_O] = np.asarray(inputs["time_features"], f32)
    pk36[0, GSB1_O:GSB1_O + G1] = np.asarray(inputs["gs_b1"], f32)
    pk36[0, GTB1_O:GTB1_O + G1] = np.asarray(inputs["gt_b1"], f32)
    pk36[0:12, WF1_O:WF1_O + F] = Wf[1]
    pk36[12, WF1_O:WF1_O + F] = bv
    pk36[0:12, WB1_O:WB1_O + F] = Wb[1]
    pk36[0:12, W01_O:W01_O + F] = Wf[0] + Wb[0]
    pk36[12, W01_O:W01_O + F] = bv
    pk36[0:12, BM_O:BM_O + F] = np.asarray(inputs["B"], f32)

    in_maps = []
    for c in range(NCORES):
        w1c = w1[:, HS * c:HS * (c + 1)]            # (8192, 256)
        w1_pack = np.ascontiguousarray(
            w1c.reshape(8, 8, 128, HS).transpose(0, 2, 1, 3)
            .reshape(8, 128, 2048)).astype(bf)
        b1c = b1[HS * c:HS * (c + 1)]
        b1T = np.repeat(b1c.reshape(2, 128).T, 2, axis=1)  # (128, 4)
        w2_pack = np.ascontiguousarray(
            w2[HS * c:HS * (c + 1)].reshape(2, 128, 2048)).astype(bf)
        # w3 cols for my nodes, permuted (node, f) -> (f, node)
        w3c = np.ascontiguousarray(
            w3[:, W3S * c:W3S * (c + 1)].reshape(2048, 128, F)
            .transpose(0, 2, 1).reshape(2048, W3S)
            .reshape(16, 128, W3S)).astype(bf)
        b3T = np.repeat(
            b3[W3S * c:W3S * (c + 1)].reshape(128, F), 2, axis=1)
        cols = (np.arange(NPC * c, NPC * (c + 1))[:, None] * F +
                np.array([9, 10, 11])[None, :]).reshape(-1)  # (384,)

        pk128 = np.zeros((128, PK128W), f32)
        pk128[:, OBS_O:OBS_O + 128] = obs_pack
        pk128[:, LI_O:LI_O + 96] = li_pack
        pk128[:, GS2_O:GS2_O + 2 * GCOLS] = \
            gs2[:, cols].reshape(2, 128, GCOLS).transpose(1, 0, 2) \
            .reshape(128, 2 * GCOLS)
        pk128[:, GT2_O:GT2_O + 2 * GCOLS] = \
            gt2[:, cols].reshape(2, 128, GCOLS).transpose(1, 0, 2) \
            .reshape(128, 2 * GCOLS)
        pk128[:, B1_O:B1_O + 4] = b1T
        pk128[:, B2_O:B2_O + 32] = b2T
        pk128[:, B3_O:B3_O + 2 * F] = b3T
        pk128[:, GS1_O:GS1_O + 2048] = gs1_pack

        pc36 = pk36.copy()
        pc36[0, GSB2_O:GSB2_O + GCOLS] = np.asarray(inputs["gs_b2"],
                                                    f32)[cols]
        pc36[0, GTB2_O:GTB2_O + GCOLS] = np.asarray(inputs["gt_b2"],
                                                    f32)[cols]
        m = {
            "pk128": pk128.astype(bf),
            "pk36": pc36.astype(bf),
            "w1_pack": w1_pack,
            "w2_pack": w2_pack,
            "w3c": w3c,
        }
        in_maps.append(m)
    return in_maps


def kernel(**inputs):
    if "nc" not in _CACHE:
        _CACHE["nc"] = _build()
    nc = _CACHE["nc"]
    in_maps = _prep_inputs(inputs)
    res = run_bass_kernel_spmd(nc, in_maps, list(range(NCORES))).results
    out0 = np.concatenate([res[c]["out0"] for c in range(NCORES)], axis=0)
    z = np.zeros((N, F), np.float32)
    return (out0, z.copy(), z.copy(), z.copy())


# revision 28
# speedup vs baseline: 1.1051x; 1.1051x over previous
"""Trainium2 Bass kernel for nn_DilatedGraphConvolutionCell.

Structural facts (derived from the reference, hardcoded):
  - conv_layer output col t=10 is the only one read by Z0[:, :, -1]; for
    dilations 3 (layers 1..3) t=10 % 3 != 0 so Z1/Z2/Z3[:, :, -1] are exact
    zeros -> outputs 1..3 are host-side zeros.
  - out0 = relu(A(10,10) @ X10 @ (Wf0+Wb0) + b)
         + relu(A(9,10) @ X9 @ Wf1 + A(11,10) @ X9 @ Wb1 + b)
    where A(a,c) = softmax(threshold(U[:,:,a] @ B @ U[:,:,c].T)) row-wise
    (degree normalization is a no-op: softmax rows sum to 1).
  - X9/X10 = rows 9,10 of the FC stack (only 2 of 12 rows needed).
  - scores are in [-1.7, 0.2] so exp() needs no max-subtraction.

v3 design: bf16 weights/activations halve HBM traffic; two collectives
(AllGather of h2-partials + U column, AllGather of X). FC layers run in
stationary-weight form so activations come out pre-transposed (no PE
transpose chains) and biases are added as multi-partition tiles (DMA cost
on TRN2 scales with bytes-per-partition, so (1, N) bias rows are slow).
fc_w3 columns are host-permuted f-major so X lands node-major for the
(E @ X) @ W tail. Threshold/exp/rowsum work overlaps the second
AllGather.

Sharding over 8 cores:
  - fc_w1 cols /8 (h1 col-shard, local), fc_w2 rows /8 (partial h2),
    fc_w3 cols /8 (X node-shard 128/core).
  - AG1 = AllGather of [h2 partial^T (128,32) | U[:, :, 10]^T (12,128)].
  - AG2 = AllGather of X^T-packed (128,24) per core.
  - adjacency + output rows node-sharded (128 rows/core).
"""

import sys

sys.path.insert(0, "/opt/trn_rl_repo")

import numpy as np
import ml_dtypes

import concourse.bass as bass
import concourse.bacc as bacc
import concourse.tile as tile
from concourse import mybir
from concourse.bass_utils import run_bass_kernel_spmd
from concourse.masks import make_identity

F32 = mybir.dt.float32
BF16 = mybir.dt.bfloat16
F8 = mybir.dt.float8e4
NPF8 = ml_dtypes.float8_e4m3
W1S, W2S, W3SC = 64.0, 32.0, 32.0  # fp8 weight scales
NPBF = ml_dtypes.bfloat16
NCORES = 8
N = 1024  # nodes
F = 12    # features (== lookback)
H = 2048  # fc hidden
HS = H // NCORES        # 256  per-core shard of fc hidden
NPC = N // NCORES       # 128  nodes per core
W3S = NPC * F           # 1536 per-core cols of fc_w3
G1 = 256                # graph-stack hidden
GCOLS = NPC * 3         # 384  per-core needed cols of gs_w2/gt_w2
OBS_O = 0
LI_O = 128
GS2_O = 224
GT2_O = 992
B1_O = 1760
B2_O = 1764
B3_O = 1796
GS1_O = 1820
PK128W = 3868
TF_O = 256
GSB1_O = 257
GSB2_O = 513
GTB1_O = 897
GTB2_O = 1153
WF1_O = 1537
WB1_O = 1549
W01_O = 1561
BM_O = 1573
B1R_O = 1585
PK36W = 1841
AF = mybir.ActivationFunctionType
ALU = mybir.AluOpType
AX = mybir.AxisListType

AG1W = 128 * 32 + F * 128  # 5632 bf16 elements per core
_CACHE = {}


def _build():
    nc = bacc.Bacc("TRN2", target_bir_lowering=False, debug=False,
                   num_devices=NCORES)
    groups = [list(range(NCORES))]

    # ---- per-core external inputs (host supplies per-core slices) ----
    d_pk128 = nc.dram_tensor("pk128", [128, PK128W], BF16,
                             kind="ExternalInput")
    d_pk36 = nc.dram_tensor("pk36", [36, PK36W], BF16, kind="ExternalInput")
    d_w1 = nc.dram_tensor("w1_pack", [8, 128, 2048], F8,
                          kind="ExternalInput")
    d_w2 = nc.dram_tensor("w2_pack", [2, 128, 2048], F8,
                          kind="ExternalInput")
    d_w3 = nc.dram_tensor("w3c", [16, 128, W3S], F8, kind="ExternalInput")
    d_b3r = nc.dram_tensor("b3row", [1, W3S], BF16, kind="ExternalInput")

    d_out = nc.dram_tensor("out0", [NPC, F], F32, kind="ExternalOutput")

    with tile.TileContext(nc) as tc:
        with (
            tc.tile_pool(name="consts", bufs=1) as consts,
            tc.tile_pool(name="wbig", bufs=1) as wbig,
            tc.tile_pool(name="w3pool", bufs=1) as w3pool,
            tc.tile_pool(name="work", bufs=2) as work,
            tc.tile_pool(name="emat", bufs=1) as emat,
            tc.tile_pool(name="ps_acc", bufs=1, space="PSUM") as ps_acc,
            tc.tile_pool(name="ps_sm", bufs=2, space="PSUM") as ps_sm,
            tc.tile_pool(name="ps_big", bufs=2, space="PSUM") as ps_big,
            tc.tile_pool(name="ps_q", bufs=3, space="PSUM") as ps_q,
            tc.tile_pool(name="dram", bufs=1, space="DRAM") as dram,
        ):
            ident_bf = consts.tile([128, 128], BF16)
            make_identity(nc, ident_bf[:])
            ones_bf = consts.tile([1, 128], BF16)
            nc.vector.memset(ones_bf[:], 1.0)
            onescol_bf = consts.tile([128, 1], BF16)
            nc.vector.memset(onescol_bf[:], 1.0)

            # ---- packed consts: two DMAs instead of ~24 -----------------
            pk128_t = consts.tile([128, PK128W], BF16)
            nc.sync.dma_start(pk128_t[:], d_pk128[:])
            pk36_t = consts.tile([36, PK36W], BF16)
            nc.sync.dma_start(pk36_t[:], d_pk36[:])
            pv = pk128_t[:]
            obs_t = pv[:, OBS_O:OBS_O + 128]
            li_t = pv[:, LI_O:LI_O + 96]
            gs2_t = [pv[:, GS2_O + GCOLS * k:GS2_O + GCOLS * (k + 1)]
                     for k in range(2)]
            gt2_t = [pv[:, GT2_O + GCOLS * k:GT2_O + GCOLS * (k + 1)]
                     for k in range(2)]
            b2T_t = pv[:, B2_O:B2_O + 32]
            gs1_t = pv[:, GS1_O:GS1_O + 2048]
            qv = pk36_t[:]
            gt1_t = qv[:, 0:G1]
            tf_t = qv[:, TF_O:TF_O + 1]
            gsb1_t = qv[0:1, GSB1_O:GSB1_O + G1]
            gsb2_t = qv[0:1, GSB2_O:GSB2_O + GCOLS]
            gtb1_t = qv[0:1, GTB1_O:GTB1_O + G1]
            gtb2_t = qv[0:1, GTB2_O:GTB2_O + GCOLS]
            b1row = qv[0:1, B1R_O:B1R_O + G1]
            Wf1b_t = qv[0:13, WF1_O:WF1_O + F]
            Wb1b_t = qv[0:13, WB1_O:WB1_O + F]
            W01b_t = qv[0:13, W01_O:W01_O + F]
            B_t = qv[0:12, BM_O:BM_O + F]
            # ---------------- graph embedding U --------------------------
            g1_ps = ps_acc.tile([F, G1], F32, tag="acc")
            for k in range(8):
                nc.tensor.matmul(g1_ps[:], li_t[:, 12 * k:12 * k + 12],
                                 gs1_t[:, 256 * k:256 * (k + 1)],
                                 start=(k == 0), stop=False)
            nc.tensor.matmul(g1_ps[:], ones_bf[:, :F], gsb1_t,
                             start=False, stop=True)
            g1_sb = work.tile([F, G1], BF16)
            nc.scalar.activation(g1_sb[:], g1_ps[:], AF.Relu)
            g1T_sb = work.tile([128, 24], BF16)
            for m in range(2):
                tp_ps = ps_sm.tile([128, F], BF16, tag="sm")
                nc.tensor.transpose(tp_ps[:], g1_sb[:, 128 * m:128 * (m + 1)],
                                    ident_bf[:F, :F])
                nc.vector.tensor_copy(g1T_sb[:, 12 * m:12 * (m + 1)], tp_ps[:])

            sp_ps = ps_acc.tile([F, GCOLS], F32, tag="acc")
            for k in range(2):
                nc.tensor.matmul(sp_ps[:], g1T_sb[:, 12 * k:12 * (k + 1)],
                                 gs2_t[k], start=(k == 0), stop=False)
            nc.tensor.matmul(sp_ps[:], ones_bf[:, :F], gsb2_t,
                             start=False, stop=True)
            sp_sb = work.tile([F, GCOLS], BF16)
            nc.scalar.activation(sp_sb[:], sp_ps[:], AF.Relu)

            t1_ps = ps_sm.tile([1, G1], F32, tag="sm")
            nc.tensor.matmul(t1_ps[:], tf_t, gt1_t, start=True,
                             stop=False)
            nc.tensor.matmul(t1_ps[:], ones_bf[:, :1], gtb1_t,
                             start=False, stop=True)
            t1_sb = work.tile([1, G1], BF16)
            nc.scalar.activation(t1_sb[:], t1_ps[:], AF.Relu)
            t1T_sb = work.tile([128, 2], BF16)
            for m in range(2):
                tt_ps = ps_sm.tile([128, 1], BF16, tag="sm")
                nc.tensor.transpose(tt_ps[:], t1_sb[:, 128 * m:128 * (m + 1)],
                                    ident_bf[:1, :1])
                nc.vector.tensor_copy(t1T_sb[:, m:m + 1], tt_ps[:])
            tp_ps2 = ps_acc.tile([1, GCOLS], F32, tag="acc")
            for k in range(2):
                nc.tensor.matmul(tp_ps2[:], t1T_sb[:, k:k + 1], gt2_t[k],
                                 start=(k == 0), stop=False)
            nc.tensor.matmul(tp_ps2[:], ones_bf[:, :1], gtb2_t,
                             start=False, stop=True)
            tp_sb = work.tile([1, GCOLS], BF16)
            nc.scalar.activation(tp_sb[:], tp_ps2[:], AF.Relu)
            tpb_ps = ps_sm.tile([F, GCOLS], F32, tag="sm")
            nc.tensor.matmul(tpb_ps[:], ones_bf[:, :F], tp_sb[:],
                             start=True, stop=True)
            U_sb = emat.tile([F, GCOLS], BF16)
            nc.vector.tensor_add(U_sb[:], sp_sb[:], tpb_ps[:])
            U_view = U_sb[:].rearrange("l (i s) -> l s i", s=3)

            # bu_s = B^T @ U1my^T for a = 9, 10, 11  (12, 128) each
            bu_sb = emat.tile([F, 3 * 128], BF16)
            for s in range(3):
                bps = ps_sm.tile([F, 128], F32, tag="sm")
                nc.tensor.matmul(bps[:], B_t, U_view[:, s, :],
                                 start=True, stop=True)
                nc.vector.tensor_copy(bu_sb[:, 128 * s:128 * (s + 1)], bps[:])

            # ---------------- fc1 (stationary w1, h1 lands transposed) ---
            h1_ps = ps_acc.tile([128, 4], F32, tag="acc")
            w1_tl = []
            for g in range(8):
                w1_t = wbig.tile([128, 2048], F8, name=f"w1t{g}")
                nc.sync.dma_start(w1_t[:], d_w1[g])
                w1_tl.append(w1_t)
            w2_t = []
            for k in range(2):
                t = consts.tile([128, 2048], F8, name=f"w2t{k}")
                nc.sync.dma_start(t[:, :1024], d_w2[k][:, :1024])
                nc.sync.dma_start(t[:, 1024:], d_w2[k][:, 1024:])
                w2_t.append(t)
            for m in range(2):
                for k in range(64):
                    g, kk = k // 8, k % 8
                    nc.tensor.matmul(
                        h1_ps[:, 2 * m:2 * m + 2],
                        w1_tl[g][:, 256 * kk + 128 * m:
                                 256 * kk + 128 * (m + 1)],
                        obs_t[:, 2 * k:2 * k + 2],
                        start=(k == 0), stop=False)
                nc.tensor.matmul(h1_ps[:, 2 * m:2 * m + 2],
                                 b1row[:, 128 * m:128 * (m + 1)],
                                 ones_bf[:, :2], start=False, stop=True)
            h1T_sb = work.tile([128, 4], BF16)
            nc.scalar.activation(h1T_sb[:], h1_ps[:], AF.Relu, scale=1.0 / W1S)

            # ---------------- fc2 partial (transposed) -------------------
            h2p_ps = ps_acc.tile([128, 32], F32, tag="acc")
            for jk in range(16):
                for k in range(2):
                    nc.tensor.matmul(
                        h2p_ps[:, 2 * jk:2 * jk + 2],
                        w2_t[k][:, 128 * jk:128 * (jk + 1)],
                        h1T_sb[:, 2 * k:2 * k + 2],
                        start=(k == 0), stop=(k == 1))
            h2pT_sb = work.tile([128, 32], BF16)
            nc.scalar.activation(h2pT_sb[:], h2p_ps[:], AF.Copy,
                                 scale=1.0 / W2S)

            # ---------------- AG1: h2 partial^T + U col 10 ---------------
            ag1_in = dram.tile([1, AG1W], BF16)
            nc.sync.dma_start(
                ag1_in[:, 4096:].rearrange("a (l i) -> l (a i)", l=F),
                U_view[:, 1, :])
            nc.sync.dma_start(
                ag1_in[:, :4096].rearrange("a (p j) -> p (a j)", p=128),
                h2pT_sb[:])
            ag1_out = dram.tile([NCORES, AG1W], BF16, addr_space="Shared")
            nc.gpsimd.collective_compute(
                "AllGather", ALU.bypass, replica_groups=groups,
                ins=[ag1_in[:].opt()], outs=[ag1_out[:].opt()])

            # ---------------- post-AG1: h2 reduce + U2T ------------------
            h2gT_sb = work.tile([128, 9 * 32], BF16)
            nc.sync.dma_start(
                h2gT_sb[:, :256].rearrange("p (c j) -> p c j", c=8),
                ag1_out[:, :4096].rearrange("c (p j) -> p c j", p=128))
            nc.sync.dma_start(h2gT_sb[:, 256:288],
                              d_pk128[:, B2_O:B2_O + 32])
            U2T_sb = emat.tile([F, N], BF16)
            nc.sync.dma_start(
                U2T_sb[:].rearrange("l (c i) -> l c i", c=NCORES),
                ag1_out[:, 4096:].rearrange("c (l i) -> l c i", l=F))

            h2r_sb = work.tile([128, 32], F32)
            nc.vector.reduce_sum(
                h2r_sb[:],
                h2gT_sb[:].rearrange("p (c j) -> p j c", c=9),
                axis=AX.X)
            h2T_sb = work.tile([128, 32], BF16)
            nc.scalar.activation(h2T_sb[:], h2r_sb[:], AF.Relu)

            # ---------------- fc3 (stationary w3, X lands node-major) ----
            # w3 cols host-permuted to (f, node): h3T psum col = 2f + t
            h3_ps = ps_acc.tile([128, 2 * F], F32, tag="acc")
            b3r_t = consts.tile([1, W3S], BF16)
            for q in range(4):
                nc.sync.dma_start(b3r_t[:, 384 * q:384 * (q + 1)],
                                  d_b3r[:, 384 * q:384 * (q + 1)])
            w3_tl = []
            for k in range(16):
                w3_t = w3pool.tile([128, W3S], F8, name=f"w3t{k}")
                nc.sync.dma_start(w3_t[:, :W3S // 2], d_w3[k][:, :W3S // 2])
                nc.sync.dma_start(w3_t[:, W3S // 2:], d_w3[k][:, W3S // 2:])
                w3_tl.append(w3_t)
            for f in range(F):
                for k in range(16):
                    nc.tensor.matmul(
                        h3_ps[:, 2 * f:2 * f + 2],
                        w3_tl[k][:, 128 * f:128 * (f + 1)],
                        h2T_sb[:, 2 * k:2 * k + 2],
                        start=(k == 0), stop=False)
                nc.tensor.matmul(h3_ps[:, 2 * f:2 * f + 2],
                                 b3r_t[:, 128 * f:128 * (f + 1)],
                                 ones_bf[:, :2], start=False, stop=True)
            h3T_sb = work.tile([128, 2 * F], BF16)
            nc.scalar.activation(h3T_sb[:], h3_ps[:], AF.Relu,
                                 scale=1.0 / W3SC)

            # ---------------- AG2: X chunks (node-major) -----------------
            ag2_in = dram.tile([1, 128 * 2 * F], BF16)
            nc.sync.dma_start(
                ag2_in[:].rearrange("a (p j) -> p (a j)", p=128),
                h3T_sb[:])
            ag2_out = dram.tile([NCORES, 128 * 2 * F], BF16,
                                addr_space="Shared")
            nc.gpsimd.collective_compute(
                "AllGather", ALU.bypass, replica_groups=groups,
                ins=[ag2_in[:].opt()], outs=[ag2_out[:].opt()])

            # ---------------- E^T + rowsums (overlap AG2) ----------------
            E_sb = []
            for s in range(3):
                e_t = emat.tile([128, N], BF16, name=f"E{s}")
                E_sb.append(e_t)
            for s in range(3):
                for hh in range(2):
                    st_ps = ps_big.tile([128, 512], F32, tag="big")
                    for j in range(4):
                        kb = 4 * hh + j
                        nc.tensor.matmul(
                            st_ps[:, 128 * j:128 * (j + 1)],
                            U2T_sb[:, 128 * kb:128 * (kb + 1)],
                            bu_sb[:, 128 * s:128 * (s + 1)],
                            start=True, stop=True)
                    msk_sb = work.tile([128, 512], F32, tag="msk")
                    nc.vector.tensor_scalar(msk_sb[:], st_ps[:], 0.05, None,
                                            op0=ALU.is_ge)
                    xt2_sb = work.tile([128, 512], F32, tag="xt2")
                    nc.vector.tensor_mul(xt2_sb[:], msk_sb[:], st_ps[:])
                    nc.scalar.activation(
                        E_sb[s][:, 512 * hh:512 * (hh + 1)], xt2_sb[:],
                        AF.Exp)
            # rowsums rs_j = sum_i E[i, j] and reciprocals
            rinv_sb = []
            for s in range(3):
                rs_ps = ps_q.tile([128, 1], F32, name=f"rs{s}", tag="q")
                for k in range(8):
                    nc.tensor.matmul(rs_ps[:],
                                     E_sb[s][:, 128 * k:128 * (k + 1)],
                                     onescol_bf[:],
                                     start=(k == 0), stop=(k == 7))
                rv = work.tile([128, 1], F32, name=f"rinv{s}")
                nc.vector.reciprocal(rv[:], rs_ps[:])
                rinv_sb.append(rv)

            # ---------------- tail: R = E @ X, out = (R/rs) @ W ----------
            X_sb = emat.tile([128, NCORES * 2 * F], BF16)
            nc.sync.dma_start(
                X_sb[:].rearrange("p (c j) -> p c j", c=NCORES),
                ag2_out[:].rearrange("c (p j) -> p c j", p=128))
            X_v = X_sb[:].rearrange("p (c f t) -> p c t f", c=NCORES, f=F)

            qnT_sb = emat.tile([13, 3 * 128], BF16)
            qnT_v = qnT_sb[:]
            for s, t in ((0, 0), (1, 1), (2, 0)):
                r_ps = ps_q.tile([128, F], F32, name=f"r{s}", tag="q")
                for k in range(8):
                    nc.tensor.matmul(r_ps[:],
                                     E_sb[s][:, 128 * k:128 * (k + 1)],
                                     X_v[:, k, t, :],
                                     start=(k == 0), stop=(k == 7))
                qn_sb = work.tile([128, F + 1], BF16, name=f"qn{s}")
                nc.vector.memset(qn_sb[:, F:F + 1], 1.0)
                nc.vector.tensor_scalar_mul(qn_sb[:, :F], r_ps[:],
                                            rinv_sb[s][:])
                qt_ps = ps_sm.tile([F + 1, 128], BF16, tag="sm")
                nc.tensor.transpose(qt_ps[:], qn_sb[:], ident_bf[:])
                nc.vector.tensor_copy(qnT_v[:, 128 * s:128 * (s + 1)],
                                      qt_ps[:])

            # bias rides as row 12 of the augmented weights
            fin_ps = ps_q.tile([128, 2 * F], F32, name="fin", tag="q")
            nc.tensor.matmul(fin_ps[:, :F], qnT_v[:, 0:128], Wf1b_t,
                             start=True, stop=False)
            nc.tensor.matmul(fin_ps[:, :F], qnT_v[:, 256:384], Wb1b_t,
                             start=False, stop=True)
            nc.tensor.matmul(fin_ps[:, F:], qnT_v[:, 128:256], W01b_t,
                             start=True, stop=True)
            fin_sb = work.tile([128, 2 * F], F32)
            nc.scalar.activation(fin_sb[:], fin_ps[:], AF.Relu)
            out_sb = work.tile([128, F], F32)
            nc.vector.tensor_add(out_sb[:], fin_sb[:, :F], fin_sb[:, F:])
            nc.sync.dma_start(d_out[:], out_sb[:])

    nc.compile()
    return nc


def _prep_inputs(inputs):
    """Host-side slicing/packing of the full inputs into per-core maps."""
    f32 = np.float32
    bf = NPBF
    obs = np.asarray(inputs["observation"], f32)
    obs2T = np.stack([obs[:, :, 9].reshape(-1), obs[:, :, 10].reshape(-1)],
                     axis=1)                       # (8192, 2)
    obs_pack = np.ascontiguousarray(
        obs2T.reshape(64, 128, 2).transpose(1, 0, 2).reshape(128, 128))
    li = np.asarray(inputs["layer_initial"], f32)   # (12, 1024)
    li_pack = np.ascontiguousarray(
        li.T.reshape(8, 128, 12).transpose(1, 0, 2).reshape(128, 96))
    gs1_pack = np.ascontiguousarray(
        np.asarray(inputs["gs_w1"], f32).reshape(8, 128, G1)
        .transpose(1, 0, 2).reshape(128, 2048))
    w1 = np.asarray(inputs["fc_w1"], f32)
    w2 = np.asarray(inputs["fc_w2"], f32)
    w3 = np.asarray(inputs["fc_w3"], f32)
    b1 = np.asarray(inputs["fc_b1"], f32)
    b2 = np.asarray(inputs["fc_b2"], f32)
    b3 = np.asarray(inputs["fc_b3"], f32)
    bv = np.asarray(inputs["b"], f32)
    gs2 = np.asarray(inputs["gs_w2"], f32)
    gt2 = np.asarray(inputs["gt_w2"], f32)
    Wf = np.asarray(inputs["W_forward"], f32)
    Wb = np.asarray(inputs["W_backward"], f32)
    b2T = np.repeat(b2.reshape(16, 128).T, 2, axis=1)  # (128, 32)

    # pk36: rows 0-35; gt1 | tf | row-0 biases | 13-row aug weights | B
    pk36 = np.zeros((36, PK36W), f32)
    pk36[:, 0:G1] = np.asarray(inputs["gt_w1"], f32)
    pk36[:, TF_O] = np.asarray(inputs["time_features"], f32)
    pk36[0, GSB1_O:GSB1_O + G1] = np.asarray(inputs["gs_b1"], f32)
    pk36[0, GTB1_O:GTB1_O + G1] = np.asarray(inputs["gt_b1"], f32)
    pk36[0:12, WF1_O:WF1_O + F] = Wf[1]
    pk36[12, WF1_O:WF1_O + F] = bv
    pk36[0:12, WB1_O:WB1_O + F] = Wb[1]
    pk36[0:12, W01_O:W01_O + F] = Wf[0] + Wb[0]
    pk36[12, W01_O:W01_O + F] = bv
    pk36[0:12, BM_O:BM_O + F] = np.asarray(inputs["B"], f32)

    in_maps = []
    for c in range(NCORES):
        w1c = w1[:, HS * c:HS * (c + 1)]            # (8192, 256)
        w1_pack = np.ascontiguousarray(
            w1c.reshape(8, 8, 128, HS).transpose(0, 2, 1, 3)
            .reshape(8, 128, 2048) * W1S).astype(NPF8)
        b1c = b1[HS * c:HS * (c + 1)]
        w2_pack = np.ascontiguousarray(
            w2[HS * c:HS * (c + 1)].reshape(2, 128, 2048) * W2S
        ).astype(NPF8)
        # w3 cols for my nodes, permuted (node, f) -> (f, node)
        w3c = np.ascontiguousarray(
            w3[:, W3S * c:W3S * (c + 1)].reshape(2048, 128, F)
            .transpose(0, 2, 1).reshape(2048, W3S)
            .reshape(16, 128, W3S) * W3SC).astype(NPF8)
        b3row = np.ascontiguousarray(
            b3[W3S * c:W3S * (c + 1)].reshape(128, F).T.reshape(1, W3S)
            * W3SC)
        cols = (np.arange(NPC * c, NPC * (c + 1))[:, None] * F +
                np.array([9, 10, 11])[None, :]).reshape(-1)  # (384,)

        pk128 = np.zeros((128, PK128W), f32)
        pk128[:, OBS_O:OBS_O + 128] = obs_pack
        pk128[:, LI_O:LI_O + 96] = li_pack
        pk128[:, GS2_O:GS2_O + 2 * GCOLS] = \
            gs2[:, cols].reshape(2, 128, GCOLS).transpose(1, 0, 2) \
            .reshape(128, 2 * GCOLS)
        pk128[:, GT2_O:GT2_O + 2 * GCOLS] = \
            gt2[:, cols].reshape(2, 128, GCOLS).transpose(1, 0, 2) \
            .reshape(128, 2 * GCOLS)
        pk128[:, B2_O:B2_O + 32] = b2T
        pk128[:, GS1_O:GS1_O + 2048] = gs1_pack

        pc36 = pk36.copy()
        pc36[0, GSB2_O:GSB2_O + GCOLS] = np.asarray(inputs["gs_b2"],
                                                    f32)[cols]
        pc36[0, GTB2_O:GTB2_O + GCOLS] = np.asarray(inputs["gt_b2"],
                                                    f32)[cols]
        pc36[0, B1R_O:B1R_O + G1] = b1c * W1S
        m = {
            "pk128": pk128.astype(bf),
            "pk36": pc36.astype(bf),
            "w1_pack": w1_pack,
            "w2_pack": w2_pack,
            "w3c": w3c,
            "b3row": b3row.astype(bf),
        }
        in_maps.append(m)
    return in_maps


def kernel(**inputs):
    if "nc" not in _CACHE:
        _CACHE["nc"] = _build()
    nc = _CACHE["nc"]
    in_maps = _prep_inputs(inputs)
    res = run_bass_kernel_spmd(nc, in_maps, list(range(NCORES))).results
    out0 = np.concatenate([res[c]["out0"] for c in range(NCORES)], axis=0)
    z = np.zeros((N, F), np.float32)
    return (out0, z.copy(), z.copy(), z.copy())
